# revision 1
# baseline (speedup 1.0000x reference)
"""Trainium2 8-core kernel for biased-attention with sigmoid gating.

Reference computation (per batch b):
  q = heads(q_x @ Wq) * C**-0.5 ; k = heads(kv_x @ Wk) ; v = heads(kv_x @ Wv)
  a = softmax(q k^T + bias1 + bias2, axis=-1)
  o = (a @ v) gated by sigmoid(q_x @ Wg + bg), then @ Wo + bo

Shapes: B=2, Q=K=2048, CQ=CK=CV=256, H=8, C=32, CO=256.

Sharding: 8 cores = 2 batches x 4 query-quarters (512 rows each). Each core
computes all 8 heads for its rows; no cross-core communication is needed.

The dominant cost is streaming the two [B,H,Q,K] f32 bias tensors (67 MB per
core, ~190 us at the ~360 GB/s per-core HBM ceiling). The kernel is built
so every engine stays under that DMA floor:
  - biases are host-transposed to [H, K, QS] so scores are produced directly
    in the transposed [k, q] orientation: no on-chip transposition of the
    8.4M-element score plane is ever needed;
  - activations/weights are host-transposed/pre-cast to bf16 (Wq carries the
    1/sqrt(C) scale), so projections start immediately;
  - per score tile: GpSimd sums b1+b2 (one pass), the PE computes QK^T
    (K=32, N=512), DVE adds the bias sum onto the PSUM result while moving
    it to SBUF, ScalarE applies exp, and the PE consumes exp(S^T) as the
    moving operand of the PV matmul;
  - V carries an extra all-ones column per head, so PV emits the softmax
    denominators for free; a tiny [33,128] PE back-transpose restores the
    natural orientation for the per-row normalization, gating, and the
    output projection.
"""

import numpy as np

B, Q, K, CQ, H, C, CO = 2, 2048, 2048, 256, 8, 32, 256
HC = H * C  # 256
QS = Q // 4  # 512 query rows per core
N_CORES = 8
SCALE = float(C) ** -0.5

_CACHED = {}


def _build():
    import concourse.bass as bass
    import concourse.mybir as mybir
    import concourse.tile as tile
    from concourse import bacc
    from concourse.masks import make_identity

    f32 = mybir.dt.float32
    bf16 = mybir.dt.bfloat16
    AF = mybir.ActivationFunctionType
    ALU = mybir.AluOpType

    nc = bacc.Bacc(None, target_bir_lowering=False)

    # activations arrive host-transposed and pre-cast to bf16: [C, rows]
    qxTd = nc.declare_dram_parameter("qxT", [CQ, QS], bf16, isOutput=False)
    kvxTd = nc.declare_dram_parameter("kvxT", [CQ, K], bf16, isOutput=False)
    # biases arrive host-transposed: [H, K, QS] (k-major), so score tiles can
    # be produced directly in the transposed [k, q] orientation
    b1 = nc.declare_dram_parameter("b1", [H, K, QS], f32, isOutput=False)
    b2 = nc.declare_dram_parameter("b2", [H, K, QS], f32, isOutput=False)
    # weights pre-cast to bf16 on host; Wq carries the C**-0.5 scale
    Wq = nc.declare_dram_parameter("Wq", [CQ, HC], bf16, isOutput=False)
    Wk = nc.declare_dram_parameter("Wk", [CQ, HC], bf16, isOutput=False)
    Wv = nc.declare_dram_parameter("Wv", [CQ, HC], bf16, isOutput=False)
    Wg = nc.declare_dram_parameter("Wg", [CQ, HC], bf16, isOutput=False)
    bg = nc.declare_dram_parameter("bg", [HC], f32, isOutput=False)
    Wo = nc.declare_dram_parameter("Wo", [HC, CO], bf16, isOutput=False)
    bo = nc.declare_dram_parameter("bo", [CO], f32, isOutput=False)
    out = nc.declare_dram_parameter("out", [QS, CO], f32, isOutput=True)

    with tile.TileContext(nc) as tc:
        with (
            tc.tile_pool(name="singles", bufs=1) as singles,
            tc.tile_pool(name="stage", bufs=3) as stage,
            tc.tile_pool(name="bias", bufs=3) as biasp,
            tc.tile_pool(name="work", bufs=3) as work,
            tc.tile_pool(name="ework", bufs=3) as ework,
            tc.tile_pool(name="ps", bufs=1, space="PSUM") as psp,
        ):
            ident = singles.tile([128, 128], bf16)
            make_identity(nc, ident)
            identf = singles.tile([128, 128], f32, tag="identf")
            make_identity(nc, identf)

            # ---- setup loads; projection critical path (kvxT, Wk, Wq, qxT)
            # issued first on the scalar ring ----
            kvxT = singles.tile([128, 2, K], bf16, tag="kvxT")
            nc.scalar.dma_start(
                out=kvxT, in_=kvxTd[:, :].rearrange("(a p) k -> p a k", p=128)
            )
            wbf = {}
            for name, w in (("Wk", Wk), ("Wq", Wq)):
                wtile = singles.tile([128, 2, 256], bf16, tag=f"w_{name}")
                nc.scalar.dma_start(
                    out=wtile, in_=w[:, :].rearrange("(a p) c -> p a c", p=128)
                )
                wbf[name] = wtile
            qxT = singles.tile([128, 2, QS], bf16, tag="qxT")
            nc.scalar.dma_start(
                out=qxT, in_=qxTd[:, :].rearrange("(a p) q -> p a q", p=128)
            )
            for name, w in (("Wv", Wv), ("Wg", Wg), ("Wo", Wo)):
                wtile = singles.tile([128, 2, 256], bf16, tag=f"w_{name}")
                nc.scalar.dma_start(
                    out=wtile, in_=w[:, :].rearrange("(a p) c -> p a c", p=128)
                )
                wbf[name] = wtile
            bg_bc = singles.tile([128, HC], f32, tag="bg")
            nc.scalar.dma_start(out=bg_bc, in_=bg[:].partition_broadcast(128))
            bo_bc = singles.tile([128, CO], f32, tag="bo")
            nc.scalar.dma_start(out=bo_bc, in_=bo[:].partition_broadcast(128))

            # Heads packed two per 128-partition tile at bases 0 and 32
            # (both legal lhsT bases); head h lives at partitions (h%2)*32
            # of pair slot h//2. Projections compute a pair per matmul (M=64).
            QT = singles.tile([128, H // 2, QS], bf16, tag="QT")
            KT = singles.tile([128, H // 2, K], bf16, tag="KT")

            def hsl(h):
                return slice((h % 2) * 32, (h % 2) * 32 + 32)

            for j in range(H // 2):
                for kc in range(4):
                    ps = psp.tile([128, 512, 1], f32, tag="scores", bufs=4)
                    for ck in range(2):
                        nc.tensor.matmul(
                            ps[:64, :, 0],
                            wbf["Wk"][:, ck, j * 64:(j + 1) * 64],
                            kvxT[:, ck, kc * 512:(kc + 1) * 512],
                            start=(ck == 0),
                            stop=(ck == 1),
                        )
                    nc.any.tensor_copy(
                        KT[:64, j, kc * 512:(kc + 1) * 512], ps[:64, :, 0]
                    )
                ps = psp.tile([128, QS, 1], f32, tag="scores", bufs=4)
                for ck in range(2):
                    nc.tensor.matmul(
                        ps[:64, :, 0],
                        wbf["Wq"][:, ck, j * 64:(j + 1) * 64],
                        qxT[:, ck, :],
                        start=(ck == 0),
                        stop=(ck == 1),
                    )
                nc.any.tensor_copy(QT[:64, j, :], ps[:64, :, 0])

            # V natural [128kr, 16kt, 8h*33] bf16; per head 32 V columns plus
            # an all-ones column so the PV matmul emits softmax denominators
            # for free in output column 32.
            Vn = singles.tile([128, K // 128, H * 33], bf16, tag="Vn")
            nc.vector.memset(Vn, 1.0)
            for kt in range(K // 128):
                ps = psp.tile([128, HC, 1], f32, tag="scores", bufs=4)
                for ck in range(2):
                    nc.tensor.matmul(
                        ps[:, :, 0],
                        kvxT[:, ck, kt * 128:(kt + 1) * 128],
                        wbf["Wv"][:, ck, :],
                        start=(ck == 0),
                        stop=(ck == 1),
                    )
                for h in range(H):
                    nc.any.tensor_copy(
                        Vn[:, kt, h * 33:h * 33 + 32], ps[:, h * 32:(h + 1) * 32, 0]
                    )

            # ---- main attention loops (transposed orientation) ----
            # Per head: stream host-transposed bias tiles B^T [128k, 512q],
            # sum them on GpSimd (bf16), add into the QK^T PSUM bank via an
            # identity matmul, exp on ScalarE straight out of PSUM, and feed
            # E^T to the PV matmul as the moving operand. Softmax denominators
            # come from V_aug's ones column; a tiny [33,128] back-transpose
            # restores natural orientation for the per-row normalization.
            O_all = singles.tile([128, 4, HC], f32, tag="O_all")
            KTILES = K // 128  # 16
            for h in range(H):
                hcol = h * 32
                o_ps = psp.tile([33, QS, 1], f32, tag="o_acc", bufs=1)
                for quarter in range(4):
                    # one 1 MB DMA per quarter-head per bias, both on the sync
                    # HWDGE ring; 4 k-tiles packed on the free dim, per-
                    # partition runs stay 2 KB contiguous rows. GpSimd sums
                    # b1+b2 (one pass); DVE adds the sum onto the QK^T PSUM
                    # result while moving it to SBUF; ScalarE applies exp;
                    # the PE only streams QK and PV.
                    B1t = biasp.tile([128, 4, QS], f32, tag="b1", bufs=7)
                    B2t = biasp.tile([128, 4, QS], f32, tag="b2", bufs=7)
                    rows = slice(quarter * 512, (quarter + 1) * 512)
                    nc.sync.dma_start(
                        out=B1t, in_=b1[h, rows, :].rearrange("(a p) q -> p a q", p=128)
                    )
                    nc.sync.dma_start(
                        out=B2t, in_=b2[h, rows, :].rearrange("(a p) q -> p a q", p=128)
                    )
                    for sub in range(4):
                        kt = quarter * 4 + sub
                        Bs = biasp.tile([128, QS], bf16, tag="bsum", bufs=6)
                        nc.gpsimd.tensor_tensor(
                            Bs, B1t[:, sub, :], B2t[:, sub, :], ALU.add
                        )
                        s_ps = psp.tile([128, QS, 1], f32, tag="scores", bufs=4)
                        nc.tensor.matmul(
                            s_ps[:, :, 0],
                            KT[hsl(h), h // 2, kt * 128:(kt + 1) * 128],
                            QT[hsl(h), h // 2, :],
                            start=True,
                            stop=True,
                        )
                        t_sb = ework.tile([128, QS], bf16, tag="t")
                        nc.vector.tensor_tensor(
                            t_sb, s_ps[:, :, 0], Bs, ALU.add
                        )
                        et_sb = ework.tile([128, QS], bf16, tag="et")
                        nc.scalar.activation(et_sb, t_sb, AF.Exp)
                        nc.tensor.matmul(
                            o_ps[:, :, 0],
                            Vn[:, kt, hcol + h:hcol + h + 33],
                            et_sb,
                            start=(kt == 0),
                            stop=(kt == KTILES - 1),
                        )
                oT_sb = work.tile([33, QS], f32, tag="oT")
                nc.vector.tensor_copy(oT_sb, o_ps[:, :, 0])
                for qt in range(4):
                    on_ps = psp.tile([128, C + 1, 1], f32, tag="onat", bufs=1)
                    nc.tensor.transpose(
                        on_ps[:, :, 0],
                        oT_sb[:, qt * 128:(qt + 1) * 128],
                        identf[:33, :33],
                    )
                    rinv = work.tile([128, 1], f32, tag="rinv")
                    nc.vector.reciprocal(rinv, on_ps[:, C:C + 1, 0])
                    nc.vector.tensor_scalar_mul(
                        O_all[:, qt, hcol:hcol + 32], on_ps[:, :C, 0], rinv
                    )

            # G natural [128q, 4qt, 256hc] f32 = sigmoid(qx @ Wg + bg)
            Gn = singles.tile([128, 4, HC], f32, tag="Gn")
            for qt in range(4):
                ps = psp.tile([128, HC, 1], f32, tag="scores", bufs=4)
                for ck in range(2):
                    nc.tensor.matmul(
                        ps[:, :, 0],
                        qxT[:, ck, qt * 128:(qt + 1) * 128],
                        wbf["Wg"][:, ck, :],
                        start=(ck == 0),
                        stop=(ck == 1),
                    )
                gt = stage.tile([128, HC], f32, tag="gtmp")
                nc.vector.tensor_add(gt, ps[:, :, 0], bg_bc)
                nc.scalar.activation(Gn[:, qt, :], gt, AF.Sigmoid)

            # ---- gating + output projection ----
            for qt in range(4):
                og = stage.tile([128, HC], bf16, tag="og")
                nc.vector.tensor_mul(og, O_all[:, qt, :], Gn[:, qt, :])
                ogt_ps = psp.tile([128, 2, 128], bf16, tag="et_ps", bufs=2)
                for hcc in range(2):
                    nc.tensor.transpose(
                        ogt_ps[:, hcc, :], og[:, hcc * 128:(hcc + 1) * 128], ident
                    )
                ogt = stage.tile([128, 2, 128], bf16, tag="ogt")
                nc.any.tensor_copy(ogt, ogt_ps)
                f_ps = psp.tile([128, CO, 1], f32, tag="scores", bufs=4)
                for hcc in range(2):
                    nc.tensor.matmul(
                        f_ps[:, :, 0],
                        ogt[:, hcc, :],
                        wbf["Wo"][:, hcc, :],
                        start=(hcc == 0),
                        stop=(hcc == 1),
                    )
                o_sb = stage.tile([128, CO], f32, tag="o_out")
                nc.vector.tensor_add(o_sb, f_ps[:, :, 0], bo_bc)
                nc.sync.dma_start(out=out[qt * 128:(qt + 1) * 128, :], in_=o_sb)

    nc.compile()
    return nc


def _get_nc():
    if "nc" not in _CACHED:
        _CACHED["nc"] = _build()
    return _CACHED["nc"]


def kernel(**inputs):
    from concourse.bass_utils import run_bass_kernel_spmd

    import ml_dtypes

    bf = ml_dtypes.bfloat16
    nc = _get_nc()
    inp = {k: np.asarray(v, dtype=np.float32) for k, v in inputs.items()}
    wq_b = (inp["Wq"] * SCALE).astype(bf)
    wk_b = inp["Wk"].astype(bf)
    wv_b = inp["Wv"].astype(bf)
    wg_b = inp["Wg"].astype(bf)
    wo_b = inp["Wo"].astype(bf)
    in_maps = []
    for c in range(N_CORES):
        b, qi = c // 4, c % 4
        q0 = qi * QS
        in_maps.append({
            "qxT": np.ascontiguousarray(inp["q_x"][b, q0:q0 + QS, :].T).astype(bf),
            "kvxT": np.ascontiguousarray(inp["kv_x"][b].T).astype(bf),
            "b1": np.ascontiguousarray(
                inp["bias1"][b, :, q0:q0 + QS, :].transpose(0, 2, 1)
            ),
            "b2": np.ascontiguousarray(
                inp["bias2"][b, :, q0:q0 + QS, :].transpose(0, 2, 1)
            ),
            "Wq": wq_b, "Wk": wk_b, "Wv": wv_b, "Wg": wg_b,
            "bg": inp["bg"], "Wo": wo_b, "bo": inp["bo"],
        })
    res = run_bass_kernel_spmd(nc, in_maps, core_ids=list(range(N_CORES)))
    outa = np.empty((B, Q, CO), np.float32)
    for c in range(N_CORES):
        b, qi = c // 4, c % 4
        outa[b, qi * QS:(qi + 1) * QS, :] = res.results[c]["out"]
    return outa



# revision 2
# speedup vs baseline: 1.0290x; 1.0290x over previous
"""Trainium2 8-core kernel for biased-attention with sigmoid gating.

Reference computation (per batch b):
  q = heads(q_x @ Wq) * C**-0.5 ; k = heads(kv_x @ Wk) ; v = heads(kv_x @ Wv)
  a = softmax(q k^T + bias1 + bias2, axis=-1)
  o = (a @ v) gated by sigmoid(q_x @ Wg + bg), then @ Wo + bo

Shapes: B=2, Q=K=2048, CQ=CK=CV=256, H=8, C=32, CO=256.

Sharding: 8 cores = 2 batches x 4 query-quarters (512 rows each). Each core
computes all 8 heads for its rows; no cross-core communication is needed.

The dominant cost is streaming the two [B,H,Q,K] bias tensors. They are
host-cast to bf16 (rel-err budget 2e-2 easily covers the ~0.2% this adds),
so each core reads 2 x 16.8 MB of bias, a ~94 us floor at the ~358 GB/s
per-core HBM ceiling. Everything else is engineered under that floor:
  - biases are host-packed to [H, 128p, 16kt*512q] so each (head, k-quarter)
    DMA is 512 KB with one contiguous 4 KB run per partition;
  - scores are produced directly in the transposed [k, q] orientation so the
    8.4M-element score plane is never transposed on chip;
  - b1+b2 presum runs on DVE at the bf16 2x rate (3 of 4 quarters) and on
    GpSimd (1 of 4) to stay under the floor on both;
  - the score+bias add alternates between the PE (identity-matmul accumulate
    into the QK^T PSUM bank, exp reads PSUM) and the DVE (PSUM add at 1x),
    splitting ~66 us of work evenly between the two engines;
  - all elementwise ops work on [128, 1024] double-k-tile slabs to amortize
    the TRN2 per-instruction fixed cost (DVE 120 cyc, ACT 172-224 cyc);
  - V carries an extra all-ones column per head so PV emits softmax
    denominators for free; a tiny [33,128] PE back-transpose restores natural
    orientation, where one fused scalar_tensor_tensor applies 1/denom and the
    sigmoid gate in a single DVE op;
  - projections for head-pairs 1..3 are deferred into the main loop so the
    first head starts attending ~6 us earlier.
"""

import numpy as np

B, Q, K, CQ, H, C, CO = 2, 2048, 2048, 256, 8, 32, 256
HC = H * C  # 256
QS = Q // 4  # 512 query rows per core
KT_N = K // 128  # 16 k-tiles
NQUARTER = H * 4  # 32 (head, k-quarter) work units
N_CORES = 8
SCALE = float(C) ** -0.5

_CACHED = {}


def _build():
    import concourse.bass as bass
    import concourse.mybir as mybir
    import concourse.tile as tile
    from concourse import bacc
    from concourse.masks import make_identity

    f32 = mybir.dt.float32
    bf16 = mybir.dt.bfloat16
    AF = mybir.ActivationFunctionType
    ALU = mybir.AluOpType

    nc = bacc.Bacc(None, target_bir_lowering=False)

    # activations arrive host-transposed and pre-cast to bf16: [C, rows]
    qxTd = nc.declare_dram_parameter("qxT", [CQ, QS], bf16, isOutput=False)
    kvxTd = nc.declare_dram_parameter("kvxT", [CQ, K], bf16, isOutput=False)
    # biases host-packed bf16 [H, 128p, 16kt*512q]: partition = k%128, free
    # dim runs over (k//128, q) so one head-quarter DMA reads 4 KB/partition
    b1d = nc.declare_dram_parameter("b1", [H, 128, KT_N * QS], bf16, isOutput=False)
    b2d = nc.declare_dram_parameter("b2", [H, 128, KT_N * QS], bf16, isOutput=False)
    # weights pre-cast to bf16 on host; Wq carries the C**-0.5 scale
    Wq = nc.declare_dram_parameter("Wq", [CQ, HC], bf16, isOutput=False)
    Wk = nc.declare_dram_parameter("Wk", [CQ, HC], bf16, isOutput=False)
    Wv = nc.declare_dram_parameter("Wv", [CQ, HC], bf16, isOutput=False)
    Wg = nc.declare_dram_parameter("Wg", [CQ, HC], bf16, isOutput=False)
    bg = nc.declare_dram_parameter("bg", [HC], f32, isOutput=False)
    Wo = nc.declare_dram_parameter("Wo", [HC, CO], bf16, isOutput=False)
    bo = nc.declare_dram_parameter("bo", [CO], f32, isOutput=False)
    out = nc.declare_dram_parameter("out", [QS, CO], f32, isOutput=True)

    with tile.TileContext(nc) as tc:
        with (
            tc.tile_pool(name="singles", bufs=1) as singles,
            tc.tile_pool(name="stage", bufs=2) as stage,
            tc.tile_pool(name="bias", bufs=1) as biasp,
            tc.tile_pool(name="work", bufs=2) as work,
            tc.tile_pool(name="ework", bufs=3) as ework,
            tc.tile_pool(name="ps", bufs=1, space="PSUM") as psp,
        ):
            # ---- setup loads; projection critical path (kvxT, Wk, Wq, qxT)
            # issued first on the scalar ring ----
            kvxT = singles.tile([128, 2, K], bf16, tag="kvxT")
            nc.scalar.dma_start(
                out=kvxT, in_=kvxTd[:, :].rearrange("(a p) k -> p a k", p=128)
            )
            wbf = {}
            for name, w in (("Wk", Wk), ("Wq", Wq)):
                wtile = singles.tile([128, 2, 256], bf16, tag=f"w_{name}")
                nc.scalar.dma_start(
                    out=wtile, in_=w[:, :].rearrange("(a p) c -> p a c", p=128)
                )
                wbf[name] = wtile
            qxT = singles.tile([128, 2, QS], bf16, tag="qxT")
            nc.scalar.dma_start(
                out=qxT, in_=qxTd[:, :].rearrange("(a p) q -> p a q", p=128)
            )
            for name, w in (("Wv", Wv), ("Wg", Wg), ("Wo", Wo)):
                wtile = singles.tile([128, 2, 256], bf16, tag=f"w_{name}")
                nc.scalar.dma_start(
                    out=wtile, in_=w[:, :].rearrange("(a p) c -> p a c", p=128)
                )
                wbf[name] = wtile
            bg_bc = singles.tile([128, HC], f32, tag="bg")
            nc.scalar.dma_start(out=bg_bc, in_=bg[:].partition_broadcast(128))
            bo_bc = singles.tile([128, CO], f32, tag="bo")
            nc.scalar.dma_start(out=bo_bc, in_=bo[:].partition_broadcast(128))

            # ---- bias streaming machinery (sync ring) ----
            # One DMA per (head, k-quarter) per bias: [128, 4*512] bf16.
            bias_tiles = {}

            def load_bias(gq):
                h, qq = divmod(gq, 4)
                sl = slice(qq * 4 * QS, (qq + 1) * 4 * QS)
                t1 = biasp.tile([128, 4 * QS], bf16, tag="b1", bufs=7, name=f"b1_{gq}")
                nc.sync.dma_start(out=t1, in_=b1d[h, :, sl])
                t2 = biasp.tile([128, 4 * QS], bf16, tag="b2", bufs=7, name=f"b2_{gq}")
                nc.sync.dma_start(out=t2, in_=b2d[h, :, sl])
                bias_tiles[gq] = (t1, t2)

            bsums = {}

            def presum(gq):
                # b1+b2 for one quarter; mostly DVE (bf16 2x), 1-in-4 GpSimd
                t1, t2 = bias_tiles.pop(gq)
                bs = biasp.tile([128, 4 * QS], bf16, tag="bs", bufs=3, name=f"bs_{gq}")
                eng = nc.gpsimd if (gq % 4 == 3) else nc.vector
                eng.tensor_tensor(bs, t1, t2, ALU.add)
                bsums[gq] = bs

            LOOK = 6
            for gq in range(LOOK):
                load_bias(gq)

            ident = singles.tile([128, 128], bf16)
            make_identity(nc, ident)
            identf = singles.tile([128, 128], f32, tag="identf")
            make_identity(nc, identf)

            # Heads packed two per 128-partition tile at bases 0 and 32
            # (both legal lhsT bases); head h lives at partitions (h%2)*32
            # of pair slot h//2. Projections compute a pair per matmul (M=64).
            QT = singles.tile([128, H // 2, QS], bf16, tag="QT")
            KT = singles.tile([128, H // 2, K], bf16, tag="KT")

            def hsl(h):
                return slice((h % 2) * 32, (h % 2) * 32 + 32)

            def proj_pair(j):
                # K/Q projections for head-pair j; copies on ScalarE so the
                # DVE stays free for bias presums.
                for kc2 in range(2):
                    ps = psp.tile([128, 2 * QS, 1], f32, tag="scores", bufs=2)
                    for c in range(2):
                        for ck in range(2):
                            nc.tensor.matmul(
                                ps[:64, c * 512:(c + 1) * 512, 0],
                                wbf["Wk"][:, ck, j * 64:(j + 1) * 64],
                                kvxT[:, ck, (kc2 * 2 + c) * 512:(kc2 * 2 + c + 1) * 512],
                                start=(ck == 0),
                                stop=(ck == 1),
                            )
                    nc.scalar.copy(KT[:64, j, kc2 * 1024:(kc2 + 1) * 1024], ps[:64, :, 0])
                ps = psp.tile([128, 2 * QS, 1], f32, tag="scores", bufs=2)
                for ck in range(2):
                    nc.tensor.matmul(
                        ps[:64, :QS, 0],
                        wbf["Wq"][:, ck, j * 64:(j + 1) * 64],
                        qxT[:, ck, :],
                        start=(ck == 0),
                        stop=(ck == 1),
                    )
                nc.scalar.copy(QT[:64, j, :], ps[:64, :QS, 0])

            proj_pair(0)

            # V natural [128kr, 16kt, 8h*33] bf16; per head 32 V columns plus
            # an all-ones column so the PV matmul emits softmax denominators
            # for free in output column 32.
            Vn = singles.tile([128, KT_N, H * 33], bf16, tag="Vn")
            nc.vector.memset(Vn, 1.0)
            for kt in range(KT_N):
                ps = psp.tile([128, 2 * QS, 1], f32, tag="scores", bufs=2)
                for ck in range(2):
                    nc.tensor.matmul(
                        ps[:, :HC, 0],
                        kvxT[:, ck, kt * 128:(kt + 1) * 128],
                        wbf["Wv"][:, ck, :],
                        start=(ck == 0),
                        stop=(ck == 1),
                    )
                nc.vector.tensor_copy(
                    Vn[:, kt, :].rearrange("p (h x) -> p h x", x=33)[:, :, :32],
                    ps[:, :HC, 0].rearrange("p (h c) -> p h c", c=32),
                )

            # G natural [128q, 4qt, 256hc] f32 = sigmoid(qx @ Wg + bg),
            # computed up-front so the tail of the kernel stays short.
            Gn = singles.tile([128, 4, HC], f32, tag="Gn")
            for qt in range(4):
                ps = psp.tile([128, 2 * QS, 1], f32, tag="scores", bufs=2)
                for ck in range(2):
                    nc.tensor.matmul(
                        ps[:, :HC, 0],
                        qxT[:, ck, qt * 128:(qt + 1) * 128],
                        wbf["Wg"][:, ck, :],
                        start=(ck == 0),
                        stop=(ck == 1),
                    )
                gt = stage.tile([128, HC], f32, tag="gt")
                nc.vector.tensor_add(gt, ps[:, :HC, 0], bg_bc)
                nc.scalar.activation(Gn[:, qt, :], gt, AF.Sigmoid)

            presum(0)
            presum(1)

            # ---- main attention loop (transposed orientation) ----
            # Per (head, k-quarter): stream bias tiles, presum them two
            # quarters ahead; per double-k-tile pair either the PE folds the
            # bias into the QK^T PSUM via an identity matmul (even pairs) or
            # the DVE adds it (odd pairs); ScalarE applies exp; the PE
            # consumes exp(S^T) as the moving operand of the PV matmul.
            O_all = singles.tile([128, 4, HC], bf16, tag="O_all")
            pair_ctr = 0
            for h in range(H):
                if 1 <= h <= 3:
                    proj_pair(h)  # overlap remaining projections with attention
                hcol = h * C
                o_ps = psp.tile([33, QS, 1], f32, tag="o_acc", bufs=2)
                for qq in range(4):
                    gq = h * 4 + qq
                    if gq + LOOK < NQUARTER:
                        load_bias(gq + LOOK)
                    if gq + 2 < NQUARTER:
                        presum(gq + 2)
                    bs = bsums.pop(gq)
                    for half in range(2):
                        pe_path = pair_ctr % 2 == 0
                        pair_ctr += 1
                        s_ps = psp.tile([128, 2 * QS, 1], f32, tag="scores", bufs=2)
                        for j in range(2):
                            lkt = half * 2 + j
                            kt = qq * 4 + lkt
                            nc.tensor.matmul(
                                s_ps[:, j * QS:(j + 1) * QS, 0],
                                KT[hsl(h), h // 2, kt * 128:(kt + 1) * 128],
                                QT[hsl(h), h // 2, :],
                                start=True,
                                stop=not pe_path,
                            )
                            if pe_path:
                                nc.tensor.matmul(
                                    s_ps[:, j * QS:(j + 1) * QS, 0],
                                    ident,
                                    bs[:, lkt * QS:(lkt + 1) * QS],
                                    start=False,
                                    stop=True,
                                    skip_group_check=True,
                                )
                        et = ework.tile([128, 2 * QS], bf16, tag="et", bufs=3)
                        if pe_path:
                            nc.scalar.activation(et, s_ps[:, :, 0], AF.Exp)
                        else:
                            tt = ework.tile([128, 2 * QS], bf16, tag="tt", bufs=3)
                            nc.vector.tensor_tensor(
                                tt,
                                s_ps[:, :, 0],
                                bs[:, half * 2 * QS:(half * 2 + 2) * QS],
                                ALU.add,
                            )
                            nc.scalar.activation(et, tt, AF.Exp)
                        for j in range(2):
                            kt = qq * 4 + half * 2 + j
                            nc.tensor.matmul(
                                o_ps[:, :, 0],
                                Vn[:, kt, h * 33:(h + 1) * 33],
                                et[:, j * QS:(j + 1) * QS],
                                start=(kt == 0),
                                stop=(kt == KT_N - 1),
                            )
                # per-head epilogue: back-transpose, normalize, gate
                oT_sb = work.tile([33, QS], f32, tag="oT", bufs=2)
                nc.scalar.copy(oT_sb, o_ps[:, :, 0])
                for qt in range(4):
                    on_ps = psp.tile([128, C + 1, 1], f32, tag="onat", bufs=1)
                    nc.tensor.transpose(
                        on_ps[:, :, 0],
                        oT_sb[:, qt * 128:(qt + 1) * 128],
                        identf[:C + 1, :C + 1],
                    )
                    rinv = work.tile([128, 1], f32, tag="rinv", bufs=2)
                    nc.vector.reciprocal(rinv, on_ps[:, C:C + 1, 0])
                    nc.vector.scalar_tensor_tensor(
                        O_all[:, qt, hcol:hcol + C],
                        on_ps[:, :C, 0],
                        rinv,
                        Gn[:, qt, hcol:hcol + C],
                        ALU.mult,
                        ALU.mult,
                    )

            # ---- output projection ----
            for qt in range(4):
                ogt_ps = psp.tile([128, 2, 128], bf16, tag="ogt", bufs=1)
                for hcc in range(2):
                    nc.tensor.transpose(
                        ogt_ps[:, hcc, :], O_all[:, qt, hcc * 128:(hcc + 1) * 128], ident
                    )
                ogt = stage.tile([128, 2, 128], bf16, tag="ogt_sb")
                nc.vector.tensor_copy(ogt, ogt_ps)
                f_ps = psp.tile([128, 2 * QS, 1], f32, tag="scores", bufs=2)
                for hcc in range(2):
                    nc.tensor.matmul(
                        f_ps[:, :CO, 0],
                        ogt[:, hcc, :],
                        wbf["Wo"][:, hcc, :],
                        start=(hcc == 0),
                        stop=(hcc == 1),
                    )
                o_sb = stage.tile([128, CO], f32, tag="o_out")
                nc.vector.tensor_add(o_sb, f_ps[:, :CO, 0], bo_bc)
                nc.sync.dma_start(out=out[qt * 128:(qt + 1) * 128, :], in_=o_sb)

    nc.compile()
    return nc


def _get_nc():
    if "nc" not in _CACHED:
        _CACHED["nc"] = _build()
    return _CACHED["nc"]


def kernel(**inputs):
    from concourse.bass_utils import run_bass_kernel_spmd

    import ml_dtypes

    bf = ml_dtypes.bfloat16
    nc = _get_nc()
    inp = {k: np.asarray(v, dtype=np.float32) for k, v in inputs.items()}
    wq_b = (inp["Wq"] * SCALE).astype(bf)
    wk_b = inp["Wk"].astype(bf)
    wv_b = inp["Wv"].astype(bf)
    wg_b = inp["Wg"].astype(bf)
    wo_b = inp["Wo"].astype(bf)

    def pack_bias(x, q0):
        # [H, Q, K] batch slice -> [H, 128p, 16kt*512q] bf16 with k = kt*128+p
        t = x[:, q0:q0 + QS, :].astype(bf)  # [H, QS, K]
        t = t.transpose(0, 2, 1)  # [H, K, QS]
        t = t.reshape(H, KT_N, 128, QS).transpose(0, 2, 1, 3)  # [H, p, kt, q]
        return np.ascontiguousarray(t).reshape(H, 128, KT_N * QS)

    in_maps = []
    for c in range(N_CORES):
        b, qi = c // 4, c % 4
        q0 = qi * QS
        in_maps.append({
            "qxT": np.ascontiguousarray(inp["q_x"][b, q0:q0 + QS, :].T).astype(bf),
            "kvxT": np.ascontiguousarray(inp["kv_x"][b].T).astype(bf),
            "b1": pack_bias(inp["bias1"][b], q0),
            "b2": pack_bias(inp["bias2"][b], q0),
            "Wq": wq_b, "Wk": wk_b, "Wv": wv_b, "Wg": wg_b,
            "bg": inp["bg"], "Wo": wo_b, "bo": inp["bo"],
        })
    res = run_bass_kernel_spmd(nc, in_maps, core_ids=list(range(N_CORES)))
    outa = np.empty((B, Q, CO), np.float32)
    for c in range(N_CORES):
        b, qi = c // 4, c % 4
        outa[b, qi * QS:(qi + 1) * QS, :] = res.results[c]["out"]
    return outa
